# revision 1
# baseline (speedup 1.0000x reference)
"""CondConv2d Trainium2 kernel.

Per-sample expert-combined 3x3 conv (B=16, 256->256 ch, 64x64, fp32),
data-parallel over batch on 8 NeuronCores (2 samples/core).

Device algorithm per core:
  1. Expert combine W_b = sum_e r_be * bank_e, with the bank pre-transposed
     host-side to [e, co_half, kh*kw, ci, co128] so combined weights land
     directly in matmul-ready [ci, co] layout. The combine is split by
     output-channel half to pipeline with the conv:
       - co-half 0 on the PE (diag(r_be).T @ bank_e accumulated in PSUM),
         hidden inside the initial bank DMA window; the diag(r_be) operands
         are built on-device from the routing row via affine_select;
       - co-half 1 on the (otherwise idle) DVE via FMA chains, hidden under
         the co-half-0 conv.
  2. Implicit-GEMM conv per co-half: out[co128, pix] accumulated over 18
     matmuls (2 ci-tiles x 9 kernel positions); rhs = shifted windows of a
     zero-padded input image in SBUF; N = 8 rows x 64 cols = 512 per matmul.
All matmuls use fp32r (FP22 multiply, full PE rate at N>=256).
"""

import os

import numpy as np

import concourse.tile as tile
from concourse import bacc, mybir
from concourse.bass_utils import run_bass_kernel_spmd

B, C_IN, C_OUT, H, W = 16, 256, 256, 64, 64
KH = KW = 3
KK = KH * KW
E = 8
N_CORES = 8
BPC = B // N_CORES  # samples per core

HP, WP = H + 2, W + 2  # zero-padded image dims
CI_T = C_IN // 128
CO_T = C_OUT // 128
KCOH = KK * 128  # per-co-half free dim of combined weights: (khkw, co128)
CCH = 3 * 128  # PE-combine chunk: 3 kernel positions x 128 co = 384
PIX_ROWS = 8  # image rows per conv matmul -> N = 8*64 = 512

F32 = mybir.dt.float32
F32R = mybir.dt.float32r
U32 = mybir.dt.uint32
Alu = mybir.AluOpType

LAST_RESULTS = None  # stashed BassKernelResults for test harness introspection
_NC_CACHE = []


def _build():
    nc = bacc.Bacc("TRN2", target_bir_lowering=False, debug=False, enable_asserts=False)
    x_d = nc.dram_tensor("x", [BPC, C_IN, H, W], F32R, kind="ExternalInput")
    bank_d = nc.dram_tensor("bank", [E, CO_T, C_IN, KK, 128], F32R, kind="ExternalInput")
    rout_d = nc.dram_tensor("rout", [128, BPC * E], F32R, kind="ExternalInput")
    out_d = nc.dram_tensor("out", [BPC, C_OUT, H, W], F32, kind="ExternalOutput")

    with tile.TileContext(nc) as tc:
        with (
            tc.tile_pool(name="const", bufs=1) as constp,
            tc.tile_pool(name="xpad", bufs=1) as xpadp,
            tc.tile_pool(name="wcomb", bufs=1) as wcombp,
            tc.tile_pool(name="bank0", bufs=6) as bank0p,
            tc.tile_pool(name="bank1", bufs=4) as bank1p,
            tc.tile_pool(name="xstg", bufs=4) as xstgp,
            tc.tile_pool(name="outs", bufs=4) as outsp,
            tc.tile_pool(name="psum", bufs=8, space="PSUM") as psump,
        ):
            # Scaled identities diag(r_be) are built on-device from the tiny
            # routing row via affine_select (value = f - p, keep on equality):
            # removes a 1MB input DMA from the critical combine window.
            # sid layout is e-major: slot i = e*BPC + b.
            rout = constp.tile([128, BPC * E], F32R, tag="rout")
            nc.sync.dma_start(rout[:], rout_d[:])
            sid = constp.tile([128, BPC * E * 128], F32R, tag="sid")
            for e in range(E):
                for b in range(BPC):
                    i = e * BPC + b
                    nc.gpsimd.affine_select(
                        sid[:, i * 128 : (i + 1) * 128],
                        rout[:, b * E + e : b * E + e + 1].broadcast_to((128, 128)),
                        pattern=[[1, 128]],
                        compare_op=Alu.is_equal,
                        fill=0.0,
                        base=0,
                        channel_multiplier=-1,
                    )

            # Zero-padded input images ([H+2, W+2] per ci-partition). Only
            # the halo is memset; the interior arrives via contiguous
            # full-rate DMA into a small staging tile + GpSimd scatter-copy
            # (strided SBUF writes are cheap on-chip, expensive for DMA).
            xpad = {}
            for b in range(BPC):
                for ct in range(CI_T):
                    t = xpadp.tile([128, HP * WP], F32R, tag=f"xpad{b}{ct}", name=f"xpad{b}{ct}")
                    u = t.bitcast(U32)
                    nc.gpsimd.memset(u[:, 0:WP], 0)  # top pad row
                    nc.gpsimd.memset(u[:, (HP - 1) * WP :], 0)  # bottom pad row
                    # side pads: pairs (row r col W+1, row r+1 col 0)
                    nc.gpsimd.memset(
                        u[:, WP - 1 : WP - 1 + 65 * WP].rearrange("p (h w) -> p h w", h=65)[:, :, 0:2],
                        0,
                    )
                    xpad[(b, ct)] = t

            # Separate combined-weight tiles per co-half: half 0 is written by
            # the ScalarE (PSUM evictions), half 1 by the DVE FMA chains.
            # Separate tiles keep the DVE stream free of false WAW deps on
            # the ScalarE evictions.
            wcomb = {}
            for b in range(BPC):
                for ct in range(CI_T):
                    for cot in range(CO_T):
                        wcomb[(b, ct, cot)] = wcombp.tile(
                            [128, KCOH], F32R, tag=f"wc{b}{ct}{cot}", name=f"wc{b}{ct}{cot}"
                        )

            # Banded input-image loads: 3 row-bands per (sample, ci-tile) so
            # they pipeline with the bank streams and unlock early conv
            # pix-groups via subtile deps.
            ROW_BANDS = [(0, 22), (22, 43), (43, 64)]

            def load_x_band(b, band):
                r0, r1 = ROW_BANDS[band]
                nrows = r1 - r0
                for ct in range(CI_T):
                    stg = xstgp.tile([128, max(r1 - r0 for r0, r1 in ROW_BANDS) * W],
                                     F32R, tag="xstg", name="xstg")
                    nc.sync.dma_start(
                        stg[:, 0 : nrows * W],
                        x_d[b, ct * 128 : (ct + 1) * 128, r0:r1, :].rearrange(
                            "ci h w -> ci (h w)"
                        ),
                    )
                    v = xpad[(b, ct)].rearrange("p (h w) -> p h w", h=HP)
                    nc.gpsimd.tensor_copy(
                        v[:, 1 + r0 : 1 + r1, 1 : W + 1],
                        stg[:, 0 : nrows * W].rearrange("p (h w) -> p h w", h=nrows),
                    )

            # ---- co-half 0 combine on the PE (streams behind bank DMA),
            #      x(b=0) bands interleaved into the same DMA window.
            # e-major: one [128, 9*128] DMA per (ct, e); 6 PSUM chunk-tiles
            # per ct accumulate across the 8 experts.
            # One early ct0 chunk-group goes to a DVE FMA chain instead of PE
            # matmuls: sim-neutral (both hide inside the bank DMA window) but
            # cuts PE-busy time on hardware with a shorter DMA window, and
            # skips its PSUM round-trip entirely. k=1 is the minimax split:
            # gate = max(window, DVE 8.9us, PE 13.6us) across plausible real
            # window lengths (k=2 makes the 17.8us DVE serial chain the gate).
            DVE_CHUNKS = {(0, 0)}
            for ct in range(CI_T):
                pe_chunks = [c for c in range(KCOH // CCH) if (ct, c) not in DVE_CHUNKS]
                pcs = {
                    (c, b): psump.tile([128, PIX_ROWS * W], F32, tag="ps", name="ps")
                    for c in pe_chunks
                    for b in range(BPC)
                }
                for e in range(E):
                    bk = bank0p.tile([128, KCOH], F32R, tag="bank0", name="bank0")
                    nc.sync.dma_start(
                        bk[:].rearrange("p (k co) -> p k co", k=KK),
                        bank_d[e, 0, ct * 128 : (ct + 1) * 128, :, :],
                    )
                    for c in pe_chunks:
                        for b in range(BPC):
                            nc.tensor.matmul(
                                pcs[(c, b)][:, 0:CCH],
                                sid[:, (e * BPC + b) * 128 : (e * BPC + b + 1) * 128],
                                bk[:, c * CCH : (c + 1) * CCH],
                                start=(e == 0),
                                stop=(e == E - 1),
                            )
                    for c in range(KCOH // CCH):
                        if (ct, c) not in DVE_CHUNKS:
                            continue
                        bslice = bk[:, c * CCH : (c + 1) * CCH]
                        for b in range(BPC):
                            wslice = wcomb[(b, ct, 0)][:, c * CCH : (c + 1) * CCH]
                            rsc = rout[:, b * E + e : b * E + e + 1].bitcast(F32)
                            if e == 0:
                                nc.vector.tensor_scalar_mul(wslice, bslice, rsc)
                            else:
                                nc.vector.scalar_tensor_tensor(
                                    wslice, bslice, rsc, wslice, Alu.mult, Alu.add
                                )
                # Evictions split across ScalarE and VectorE so the serial
                # tail after the last expert's matmuls is halved.
                for c in pe_chunks:
                    for b in range(BPC):
                        dst = wcomb[(b, ct, 0)][:, c * CCH : (c + 1) * CCH]
                        srcp = pcs[(c, b)][:, 0:CCH]
                        if (c + b) % 2 == 0:
                            nc.scalar.copy(dst, srcp)
                        else:
                            nc.vector.tensor_copy(dst, srcp)

            # ---- co-half 1 combine on the DVE (hidden under co-half 0 conv);
            #      x(b=1) bands interleaved mid-stream ----
            def combine_dve(ct):
                for e in range(E):
                    bk = bank1p.tile([128, KCOH], F32R, tag="bank1", name="bank1")
                    nc.sync.dma_start(
                        bk[:].rearrange("p (k co) -> p k co", k=KK),
                        bank_d[e, 1, ct * 128 : (ct + 1) * 128, :, :],
                    )
                    for b in range(BPC):
                        wslice = wcomb[(b, ct, 1)][:]
                        rsc = rout[:, b * E + e : b * E + e + 1].bitcast(F32)
                        if e == 0:
                            nc.vector.tensor_scalar_mul(wslice, bk[:], rsc)
                        else:
                            nc.vector.scalar_tensor_tensor(
                                wslice, bk[:], rsc, wslice, Alu.mult, Alu.add
                            )

            load_x_band(0, 0)
            load_x_band(0, 1)
            load_x_band(0, 2)
            combine_dve(0)
            load_x_band(1, 0)
            load_x_band(1, 1)
            load_x_band(1, 2)
            combine_dve(1)

            # ---- conv as implicit GEMM, co-half major ----
            for cot in range(CO_T):
                for b in range(BPC):
                    for p in range(H // PIX_ROWS):
                        h0 = p * PIX_ROWS
                        pc = psump.tile([128, PIX_ROWS * W], F32, tag="ps", name="ps")
                        first = True
                        for ct in range(CI_T):
                            xv = xpad[(b, ct)].rearrange("p (h w) -> p h w", h=HP)
                            for kh in range(KH):
                                for kw in range(KW):
                                    kk = kh * KW + kw
                                    lhsT = wcomb[(b, ct, cot)][
                                        :, kk * 128 : (kk + 1) * 128
                                    ]
                                    rhs = xv[:, h0 + kh : h0 + kh + PIX_ROWS, kw : kw + W]
                                    last = ct == CI_T - 1 and kk == KK - 1
                                    nc.tensor.matmul(pc[:], lhsT, rhs, start=first, stop=last)
                                    first = False
                        ot = outsp.tile([128, PIX_ROWS * W], F32, tag="outs", name="outs")
                        nc.scalar.copy(ot[:], pc[:])
                        nc.sync.dma_start(
                            out_d[b, cot * 128 : (cot + 1) * 128, h0 : h0 + PIX_ROWS, :],
                            ot.rearrange("p (h w) -> p h w", h=PIX_ROWS),
                        )
    nc.compile()
    return nc


def kernel(x, routing_weights, expert_weight):
    global LAST_RESULTS
    x = np.ascontiguousarray(np.asarray(x, dtype=np.float32))
    r = np.asarray(routing_weights, dtype=np.float32)
    bank = np.asarray(expert_weight, dtype=np.float32)

    # Host relayout: [e, co*ci*kh*kw] -> [e, co_half, kh*kw, ci, co128] so
    # combined weights come out of the device combine in matmul-ready
    # [ci, co] tiles, co-half major.
    bank_t = np.ascontiguousarray(
        bank.reshape(E, CO_T, 128, C_IN, KK).transpose(0, 1, 3, 4, 2)
    )

    if not _NC_CACHE:
        _NC_CACHE.append(_build())
    nc = _NC_CACHE[0]

    in_maps = []
    for c in range(N_CORES):
        rows = r[c * BPC : (c + 1) * BPC].reshape(BPC * E)
        in_maps.append(
            {
                "x": np.ascontiguousarray(x[c * BPC : (c + 1) * BPC]),
                "bank": bank_t,
                "rout": np.ascontiguousarray(
                    np.broadcast_to(rows[None, :], (128, BPC * E))
                ),
            }
        )

    trace = bool(os.environ.get("KERNEL_TRACE"))
    try:
        res = run_bass_kernel_spmd(
            nc, in_maps, core_ids=list(range(N_CORES)), trace=trace
        )
    except ModuleNotFoundError:
        if not trace:
            raise
        # Tracing unavailable in this environment (no axon NTFF hook).
        res = run_bass_kernel_spmd(
            nc, in_maps, core_ids=list(range(N_CORES)), trace=False
        )
    LAST_RESULTS = res
    return np.concatenate([rr["out"] for rr in res.results], axis=0)



# revision 6
# speedup vs baseline: 1.3011x; 1.3011x over previous
"""CondConv2d Trainium2 kernel — fp8 DoubleRow implicit GEMM.

Per-sample expert-combined 3x3 conv (B=16, 256->256 ch, 64x64, fp32),
data-parallel over batch on 8 NeuronCores (2 samples/core).

Math: out_b = conv(x_b, W_b), W_b = sum_e r_be Bank_e. All MACs run on the
PE in fp8e4m3 DoubleRow mode (2 K-tiles per matmul, 2 cols/cycle) with a
3-term error-compensated decomposition:

    x = x8 + x8r   (fp8 value + fp8 residual, host-side quantization)
    W = w8 + w8r   (fp8 split computed on device from the combined weights)
    out ~= x8*w8 + x8r*w8 + x8*w8r      (dropped term x8r*w8r ~ 0.2%)

Per 8-row pixel group: 27 DoubleRow matmuls (9 kernel taps x 3 terms, each
contracting both 128-channel tiles at once), N=512 at 0.5 cycles/row.

Device schedule per core:
  - Expert combine W_b = sum_e r_be Bank_e in bf16/fp32:
      * startup-critical chunk (b0, cot0, taps 0-2) via diag(r) matmuls on
        the otherwise-idle PE (doubles as the p-state warmup), split to
        w8/w8r straight out of PSUM;
      * everything else on the DVE as 8 tensor_scalar muls (4x bf16 mode)
        + a 7-add tree (2x bf16 mode), then fp8 split (w8 on Act, w8r on
        DVE).
  - x arrives pre-padded/pre-split from the host as fp8 [ci128, ct2, 66, 66]
    so no on-device padding or scatter work exists.
  - First conv group runs as 3 tap-chunk passes over 6 PSUM banks so it
    starts when only taps 0-2 are combined; later groups are single-pass.
  - Output: PSUM -> SBUF eviction and the out-DMA both on the Act engine
    (keeps the SP queue free for the input stream).
"""

import os

import numpy as np
import ml_dtypes

import concourse.tile as tile
from concourse import bacc, mybir
from concourse.bass_utils import run_bass_kernel_spmd

B, C_IN, C_OUT, H, W = 16, 256, 256, 64, 64
KH = KW = 3
KK = KH * KW
E = 8
N_CORES = 8
BPC = B // N_CORES  # samples per core

HP, WP = H + 2, W + 2  # zero-padded image dims
CT = 2  # ci tiles (DoubleRow K-tile pairs)
CO_T = 2  # co tiles (output-channel halves)
NCH = 3  # kk chunks
CHK = 3  # taps per chunk
CCH = CHK * 128  # chunk free size in weight layout
PIX_ROWS = 8  # image rows per conv matmul -> N = 8*64 = 512
NPG = H // PIX_ROWS  # pixel groups per (b, cot)

# Combined weights are ~1e-2 — inside fp8e4m3's subnormal range, where the
# residual split can't resolve anything. Scale the bank into the normal
# range host-side and descale in the eviction copy (power of two: exact).
W_SCALE = 512.0

F32 = mybir.dt.float32
BF16 = mybir.dt.bfloat16
F8 = mybir.dt.float8e4
Alu = mybir.AluOpType
DR = mybir.MatmulPerfMode.DoubleRow

X_BANDS = [(0, 34), (34, 50), (50, 66)]  # padded-row bands for sample 0

LAST_RESULTS = None  # stashed BassKernelResults for test harness introspection
_NC_CACHE = []


def _build():
    nc = bacc.Bacc("TRN2", target_bir_lowering=False, debug=False, enable_asserts=False)
    x8_d = nc.dram_tensor("x8", [BPC, 128, CT * HP * WP], F8, kind="ExternalInput")
    x8r_d = nc.dram_tensor("x8r", [BPC, 128, CT * HP * WP], F8, kind="ExternalInput")
    # [cot, ct, chunk, e, ci, chunk-taps*co]
    bank_d = nc.dram_tensor("bank", [CO_T, CT, NCH, E, 128, CCH], BF16, kind="ExternalInput")
    routf_d = nc.dram_tensor("routf", [128, BPC * E], F32, kind="ExternalInput")
    routb_d = nc.dram_tensor("routb", [128, BPC * E], BF16, kind="ExternalInput")
    out_d = nc.dram_tensor("out", [BPC, C_OUT, H, W], F32, kind="ExternalOutput")

    with tile.TileContext(nc) as tc:
        with (
            tc.tile_pool(name="const", bufs=1) as constp,
            tc.tile_pool(name="xp", bufs=1) as xpp,
            tc.tile_pool(name="bank", bufs=1) as bankp,
            tc.tile_pool(name="w8", bufs=1) as w8p,
            tc.tile_pool(name="scr", bufs=1) as scrp,
            tc.tile_pool(name="outs", bufs=6) as outsp,
            tc.tile_pool(name="cpsum", bufs=2, space="PSUM") as cpsump,
            tc.tile_pool(name="psum", bufs=6, space="PSUM") as psump,
        ):
            routf = constp.tile([128, BPC * E], F32, tag="routf")
            routb = constp.tile([128, BPC * E], BF16, tag="routb")
            nc.sync.dma_start(routf[:], routf_d[:])
            nc.sync.dma_start(routb[:], routb_d[:])

            # diag(r_b0,e) tiles for the PE-side startup combine (sample 0)
            sid = constp.tile([128, E * 128], BF16, tag="sid")
            for e in range(E):
                nc.gpsimd.affine_select(
                    sid[:, e * 128 : (e + 1) * 128],
                    routb[:, e : e + 1].broadcast_to((128, 128)),
                    pattern=[[1, 128]],
                    compare_op=Alu.is_equal,
                    fill=0.0,
                    base=0,
                    channel_multiplier=-1,
                )

            # ---- SBUF tiles ----
            x8 = {}
            x8r = {}
            for b in range(BPC):
                x8[b] = xpp.tile([128, CT * HP * WP], F8, tag=f"x8_{b}", name=f"x8_{b}")
                x8r[b] = xpp.tile([128, CT * HP * WP], F8, tag=f"x8r_{b}", name=f"x8r_{b}")

            # bank tile per (ct, cot): [ci, chunk, e, CCH]
            bkt = {}
            for cot in range(CO_T):
                for ct in range(CT):
                    bkt[(ct, cot)] = bankp.tile(
                        [128, NCH * E * CCH], BF16, tag=f"bk{ct}{cot}", name=f"bk{ct}{cot}"
                    )

            # w8 / w8r per (b, cot): [ci, kk, ct, co]
            w8 = {}
            w8r = {}
            for b in range(BPC):
                for cot in range(CO_T):
                    w8[(b, cot)] = w8p.tile(
                        [128, KK * CT * 128], F8, tag=f"w8_{b}{cot}", name=f"w8_{b}{cot}"
                    )
                    w8r[(b, cot)] = w8p.tile(
                        [128, KK * CT * 128], F8, tag=f"w8r_{b}{cot}", name=f"w8r_{b}{cot}"
                    )

            def w8v(t):
                return t.rearrange("p (k c o) -> p k c o", k=KK, c=CT)

            # DVE combine scratch: per-expert products + add tree + result
            s = {e: scrp.tile([128, KK * 128], BF16, tag=f"s{e}", name=f"s{e}") for e in range(E)}
            t4 = {i: scrp.tile([128, KK * 128], BF16, tag=f"t{i}", name=f"t{i}") for i in range(4)}
            u2 = {i: scrp.tile([128, KK * 128], BF16, tag=f"u{i}", name=f"u{i}") for i in range(2)}
            vres = scrp.tile([128, KK * 128], BF16, tag="v", name="v")

            # ---- input DMA stream (SP queue, priority order) ----
            def dma_bank(ct, cot, c):
                dst = bkt[(ct, cot)].rearrange("p (c e f) -> p c e f", c=NCH, e=E)
                nc.sync.dma_start(
                    dst[:, c : c + 1, :, :],
                    bank_d[cot, ct, c : c + 1, :, :, :].rearrange("c e ci f -> ci c e f"),
                )

            def dma_x_band(b, band):
                r0, r1 = X_BANDS[band]
                for t, td in ((x8[b], x8_d), (x8r[b], x8r_d)):
                    v = t.rearrange("p (c h w) -> p c h w", c=CT, h=HP)
                    nc.sync.dma_start(
                        v[:, :, r0:r1, :],
                        td[b].rearrange("p (c h w) -> p c h w", c=CT, h=HP)[:, :, r0:r1, :],
                    )

            dma_bank(0, 0, 0)
            dma_bank(1, 0, 0)
            dma_x_band(0, 0)
            dma_bank(0, 0, 1)
            dma_bank(1, 0, 1)
            dma_x_band(0, 1)
            dma_bank(0, 0, 2)
            dma_bank(1, 0, 2)
            dma_x_band(0, 2)
            for t, td in ((x8[1], x8_d), (x8r[1], x8r_d)):
                nc.sync.dma_start(t[:], td[1])
            for c in range(NCH):
                dma_bank(0, 1, c)
                dma_bank(1, 1, c)

            # ---- startup combine on PE: (b0, cot0, chunk 0), both ct ----
            # diag(r) matmuls accumulate sum_e r_e * bank_e in PSUM; Act
            # converts PSUM->w8 (fp8), DVE computes w8r = psum - w8.
            for ct in range(CT):
                ps = cpsump.tile([128, CCH], F32, tag="cps", name="cps")
                bview = bkt[(ct, 0)].rearrange("p (c e f) -> p c e f", c=NCH, e=E)
                for e in range(E):
                    nc.tensor.matmul(
                        ps[:],
                        sid[:, e * 128 : (e + 1) * 128],
                        bview[:, 0, e, :],
                        start=(e == 0),
                        stop=(e == E - 1),
                    )
                psv = ps.rearrange("p (k o) -> p k o", k=CHK)
                nc.scalar.copy(w8v(w8[(0, 0)])[:, 0:CHK, ct, :], psv[:])
                nc.vector.tensor_tensor(
                    w8v(w8r[(0, 0)])[:, 0:CHK, ct, :],
                    psv[:],
                    w8v(w8[(0, 0)])[:, 0:CHK, ct, :],
                    Alu.subtract,
                )

            # ---- DVE combine for one (b, ct, cot, chunk-range) ----
            def combine_dve(b, ct, cot, c0, c1):
                """Combine taps [3*c0, 3*c1) of (b, ct, cot) on the DVE."""
                k0, k1 = CHK * c0, CHK * c1
                n = (k1 - k0) * 128
                bview = bkt[(ct, cot)].rearrange("p (c e f) -> p c e f", c=NCH, e=E)

                def sl(tile_):
                    return tile_[:, k0 * 128 : k1 * 128].rearrange(
                        "p (k o) -> p k o", k=k1 - k0
                    )

                rsc = routf[:, b * E : b * E + E]
                for e in range(E):
                    nc.vector.tensor_scalar_mul(
                        sl(s[e]), bview[:, c0:c1, e, :], rsc[:, e : e + 1]
                    )
                for i in range(4):
                    nc.vector.tensor_tensor(
                        sl(t4[i]), sl(s[2 * i]), sl(s[2 * i + 1]), Alu.add
                    )
                for i in range(2):
                    nc.vector.tensor_tensor(
                        sl(u2[i]), sl(t4[2 * i]), sl(t4[2 * i + 1]), Alu.add
                    )
                nc.vector.tensor_tensor(sl(vres), sl(u2[0]), sl(u2[1]), Alu.add)
                nc.scalar.copy(w8v(w8[(b, cot)])[:, k0:k1, ct, :], sl(vres))
                nc.vector.tensor_tensor(
                    w8v(w8r[(b, cot)])[:, k0:k1, ct, :],
                    sl(vres),
                    w8v(w8[(b, cot)])[:, k0:k1, ct, :],
                    Alu.subtract,
                )

            # chunk-granular for the startup group, full-unit for the rest
            combine_dve(0, 0, 0, 1, 2)
            combine_dve(0, 1, 0, 1, 2)
            combine_dve(0, 0, 0, 2, 3)
            combine_dve(0, 1, 0, 2, 3)
            combine_dve(1, 0, 0, 0, 3)
            combine_dve(1, 1, 0, 0, 3)
            combine_dve(0, 0, 1, 0, 3)
            combine_dve(0, 1, 1, 0, 3)
            combine_dve(1, 0, 1, 0, 3)
            combine_dve(1, 1, 1, 0, 3)

            # ---- conv: fp8 DoubleRow implicit GEMM ----
            def mm(pc, b, cot, p, kk, term, start, stop):
                kh, kw = kk // KW, kk % KW
                wt = w8[(b, cot)] if term < 2 else w8r[(b, cot)]
                xt = x8[b] if term != 1 else x8r[b]
                xv = xt.rearrange("p (c h w) -> p c h w", c=CT, h=HP)
                nc.tensor.matmul(
                    pc[:],
                    w8v(wt)[:, kk, :, :],
                    xv[:, :, p * PIX_ROWS + kh : p * PIX_ROWS + kh + PIX_ROWS, kw : kw + W],
                    start=start,
                    stop=stop,
                    perf_mode=DR,
                    skip_group_check=True,
                )

            def evict(pc, b, cot, p):
                ot = outsp.tile([128, PIX_ROWS * W], F32, tag="outs", name="outs")
                nc.scalar.mul(ot[:], pc[:], 1.0 / W_SCALE)
                nc.scalar.dma_start(
                    out_d[b, cot * 128 : (cot + 1) * 128, p * PIX_ROWS : (p + 1) * PIX_ROWS, :],
                    ot.rearrange("p (h w) -> p h w", h=PIX_ROWS),
                )

            # group (b0, cot0): 3 chunk-passes over p-groups 0-5, then 6,7
            NP_PASS = 6
            pcs = {p: psump.tile([128, PIX_ROWS * W], F32, tag="ps", name="ps") for p in range(NP_PASS)}
            for c in range(NCH):
                for p in range(NP_PASS):
                    for kk in range(CHK * c, CHK * (c + 1)):
                        for term in range(3):
                            mm(
                                pcs[p], 0, 0, p, kk, term,
                                start=(c == 0 and kk == 0 and term == 0),
                                stop=(c == NCH - 1 and kk == KK - 1 and term == 2),
                            )
            for p in range(NP_PASS):
                evict(pcs[p], 0, 0, p)

            def conv_group(b, cot, p_range):
                for p in p_range:
                    pc = psump.tile([128, PIX_ROWS * W], F32, tag="ps", name="ps")
                    first = True
                    for kk in range(KK):
                        for term in range(3):
                            last = kk == KK - 1 and term == 2
                            mm(pc, b, cot, p, kk, term, start=first, stop=last)
                            first = False
                    evict(pc, b, cot, p)

            conv_group(0, 0, range(NP_PASS, NPG))
            conv_group(1, 0, range(NPG))
            conv_group(0, 1, range(NPG))
            conv_group(1, 1, range(NPG))
    nc.compile()
    return nc


def kernel(x, routing_weights, expert_weight):
    global LAST_RESULTS
    x = np.asarray(x, dtype=np.float32)
    r = np.asarray(routing_weights, dtype=np.float32)
    bank = np.asarray(expert_weight, dtype=np.float32)
    fp8 = ml_dtypes.float8_e4m3

    # x: [B, 256, H, W] -> padded per-ct layout [B, 128, ct, HP, WP],
    # split into fp8 value + fp8 residual (host-side quantization).
    xp = np.zeros((B, 128, CT, HP, WP), dtype=np.float32)
    xp[:, :, :, 1 : H + 1, 1 : W + 1] = x.reshape(B, CT, 128, H, W).transpose(0, 2, 1, 3, 4)
    x8 = xp.astype(fp8)
    x8r = (xp - x8.astype(np.float32)).astype(fp8)
    x8 = x8.reshape(B, 128, CT * HP * WP)
    x8r = x8r.reshape(B, 128, CT * HP * WP)

    # bank: [E, co*ci*kh*kw] -> [cot, ct, chunk, e, ci, (3 taps, co)] bf16
    bk = bank.reshape(E, CO_T, 128, CT, 128, NCH, CHK) * W_SCALE
    bank_t = np.ascontiguousarray(
        bk.transpose(1, 3, 5, 0, 4, 6, 2).reshape(CO_T, CT, NCH, E, 128, CCH)
    ).astype(ml_dtypes.bfloat16)

    if not _NC_CACHE:
        _NC_CACHE.append(_build())
    nc = _NC_CACHE[0]

    in_maps = []
    for c in range(N_CORES):
        rows = r[c * BPC : (c + 1) * BPC].reshape(BPC * E)
        rbc = np.ascontiguousarray(np.broadcast_to(rows[None, :], (128, BPC * E)))
        in_maps.append(
            {
                "x8": np.ascontiguousarray(x8[c * BPC : (c + 1) * BPC]),
                "x8r": np.ascontiguousarray(x8r[c * BPC : (c + 1) * BPC]),
                "bank": bank_t,
                "routf": rbc,
                "routb": rbc.astype(ml_dtypes.bfloat16),
            }
        )

    trace = bool(os.environ.get("KERNEL_TRACE"))
    try:
        res = run_bass_kernel_spmd(
            nc, in_maps, core_ids=list(range(N_CORES)), trace=trace
        )
    except ModuleNotFoundError:
        if not trace:
            raise
        # Tracing unavailable in this environment (no axon NTFF hook).
        res = run_bass_kernel_spmd(
            nc, in_maps, core_ids=list(range(N_CORES)), trace=False
        )
    LAST_RESULTS = res
    return np.concatenate([rr["out"] for rr in res.results], axis=0)


# revision 9
# speedup vs baseline: 1.4180x; 1.0899x over previous
"""CondConv2d Trainium2 kernel — fp8 DoubleRow implicit GEMM.

Per-sample expert-combined 3x3 conv (B=16, 256->256 ch, 64x64, fp32),
data-parallel over batch on 8 NeuronCores (2 samples/core).

Math: out_b = conv(x_b, W_b), W_b = sum_e r_be Bank_e. All MACs run on the
PE in fp8e4m3 DoubleRow mode (2 K-tiles per matmul, 2 cols/cycle) with a
3-term error-compensated decomposition:

    x = x8 + x8r     (fp8 value + fp8 residual, host-side quantization)
    W*S = w8 + w8r   (fp8 split computed on device; S=512 lifts the tiny
                      combined weights out of fp8's subnormal range, and
                      the eviction copy descales by 1/S — exact, power of 2)
    out ~= (x8*w8 + x8r*w8 + x8*w8r) / S    (dropped term x8r*w8r ~ 0.1%)

Per 8-row pixel group: 27 DoubleRow matmuls (9 taps x 3 terms, each
contracting both 128-channel tiles at once), N=512 at 0.5 cycles/row.

Device schedule per core (engines pipelined, all queues in-order):
  - PE: 6 garbage warmup matmuls (p-state ramp) -> diag(r) combine of
    (b0, cot0) one 3-tap chunk at a time, each chunk followed by a conv
    pass over pixel groups 0-3 consuming just that chunk -> single-pass
    pixel groups 4-7 -> remaining three (b, cot) groups single-pass.
  - DVE: b1/cot0 products chunk-wise as bank chunks land, w8r splits for
    the PE-combined chunks, then full-unit combines (8 muls in 4x bf16
    mode + 7-add tree in 2x mode) for the remaining (b, ct, cot) units.
  - Act: w8 fp8 splits + PSUM evictions (with 1/S descale) + output DMAs,
    emission-ordered so no split head-blocks an eviction.
  - x arrives pre-padded/pre-split from the host as fp8 [ci128, ct2, 66, 66]
    in 4 row-bands for sample 0; no on-device padding or scatter work.
  - Last pixel group split in two 4-row halves to shorten the drain tail.
"""

import os

import numpy as np
import ml_dtypes

import concourse.tile as tile
from concourse import bacc, mybir
from concourse.bass_utils import run_bass_kernel_spmd

B, C_IN, C_OUT, H, W = 16, 256, 256, 64, 64
KH = KW = 3
KK = KH * KW
E = 8
N_CORES = 8
BPC = B // N_CORES  # samples per core

HP, WP = H + 2, W + 2  # zero-padded image dims
CT = 2  # ci tiles (DoubleRow K-tile pairs)
CO_T = 2  # co tiles (output-channel halves)
NCH = 3  # kk chunks
CHK = 3  # taps per chunk
CCH = CHK * 128  # chunk free size in weight layout
PIX_ROWS = 8  # image rows per conv matmul -> N = 8*64 = 512
NPG = H // PIX_ROWS  # pixel groups per (b, cot)
NP_PASS = 4  # pixel groups covered by the chunk-pass window

# Combined weights are ~1e-2 — inside fp8e4m3's subnormal range, where the
# residual split can't resolve anything. Scale the bank into the normal
# range host-side and descale in the eviction copy (power of two: exact).
W_SCALE = 512.0

F32 = mybir.dt.float32
BF16 = mybir.dt.bfloat16
F8 = mybir.dt.float8e4
Alu = mybir.AluOpType
DR = mybir.MatmulPerfMode.DoubleRow

X_BANDS = [(0, 18), (18, 34), (34, 50), (50, 66)]  # padded-row bands, sample 0

LAST_RESULTS = None  # stashed BassKernelResults for test harness introspection
_NC_CACHE = []


def _build():
    nc = bacc.Bacc("TRN2", target_bir_lowering=False, debug=False, enable_asserts=False)
    x8_d = nc.dram_tensor("x8", [BPC, 128, CT * HP * WP], F8, kind="ExternalInput")
    x8r_d = nc.dram_tensor("x8r", [BPC, 128, CT * HP * WP], F8, kind="ExternalInput")
    # [cot, ct, chunk, e, ci, chunk-taps*co]
    bank_d = nc.dram_tensor("bank", [CO_T, CT, NCH, E, 128, CCH], BF16, kind="ExternalInput")
    routb_d = nc.dram_tensor("routb", [128, BPC * E], BF16, kind="ExternalInput")
    routf_d = nc.dram_tensor("routf", [128, BPC * E], F32, kind="ExternalInput")
    out_d = nc.dram_tensor("out", [BPC, C_OUT, H, W], F32, kind="ExternalOutput")

    with tile.TileContext(nc) as tc:
        with (
            tc.tile_pool(name="const", bufs=1) as constp,
            tc.tile_pool(name="xp", bufs=1) as xpp,
            tc.tile_pool(name="bank", bufs=1) as bankp,
            tc.tile_pool(name="w8", bufs=1) as w8p,
            tc.tile_pool(name="scr", bufs=1) as scrp,
            tc.tile_pool(name="outs", bufs=6) as outsp,
            tc.tile_pool(name="cpsum", bufs=2, space="PSUM") as cpsump,
            tc.tile_pool(name="psum", bufs=6, space="PSUM") as psump,
        ):
            routb = constp.tile([128, BPC * E], BF16, tag="routb")
            routf = constp.tile([128, BPC * E], F32, tag="routf")
            nc.sync.dma_start(routb[:], routb_d[:])
            nc.sync.dma_start(routf[:], routf_d[:])

            # warmup scratch (zeros) for p-state ramp matmuls
            wtile = constp.tile([128, 512], F8, tag="warm")
            nc.gpsimd.memset(wtile[:], 0)

            # diag(r_b0,e) tiles for the PE-side startup combine (sample 0)
            sid = constp.tile([128, E * 128], BF16, tag="sid")
            for e in range(E):
                nc.gpsimd.affine_select(
                    sid[:, e * 128 : (e + 1) * 128],
                    routb[:, e : e + 1].broadcast_to((128, 128)),
                    pattern=[[1, 128]],
                    compare_op=Alu.is_equal,
                    fill=0.0,
                    base=0,
                    channel_multiplier=-1,
                )

            # ---- SBUF tiles ----
            x8 = {}
            x8r = {}
            for b in range(BPC):
                x8[b] = xpp.tile([128, CT * HP * WP], F8, tag=f"x8_{b}", name=f"x8_{b}")
                x8r[b] = xpp.tile([128, CT * HP * WP], F8, tag=f"x8r_{b}", name=f"x8r_{b}")

            bkt = {}
            for cot in range(CO_T):
                for ct in range(CT):
                    bkt[(ct, cot)] = bankp.tile(
                        [128, NCH * E * CCH], BF16, tag=f"bk{ct}{cot}", name=f"bk{ct}{cot}"
                    )

            def bkv(ct, cot):
                return bkt[(ct, cot)].rearrange("p (c e f) -> p c e f", c=NCH, e=E)

            w8 = {}
            w8r = {}
            for b in range(BPC):
                for cot in range(CO_T):
                    w8[(b, cot)] = w8p.tile(
                        [128, KK * CT * 128], F8, tag=f"w8_{b}{cot}", name=f"w8_{b}{cot}"
                    )
                    w8r[(b, cot)] = w8p.tile(
                        [128, KK * CT * 128], F8, tag=f"w8r_{b}{cot}", name=f"w8r_{b}{cot}"
                    )

            def w8v(t):
                return t.rearrange("p (k c o) -> p k c o", k=KK, c=CT)

            # DVE combine scratch: per-expert products + add tree + results
            s = {e: scrp.tile([128, KK * 128], BF16, tag=f"s{e}", name=f"s{e}") for e in range(E)}
            t4 = {i: scrp.tile([128, KK * 128], BF16, tag=f"t{i}", name=f"t{i}") for i in range(4)}
            u2 = {i: scrp.tile([128, KK * 128], BF16, tag=f"u{i}", name=f"u{i}") for i in range(2)}
            vres = {c: scrp.tile([128, KK * 128], BF16, tag=f"v{c}", name=f"v{c}") for c in range(CT)}

            # ---- input DMA stream (SP queue, priority order) ----
            def dma_bank(ct, cot, c, ehalf=None):
                dst = bkv(ct, cot)
                e0, e1 = (0, E) if ehalf is None else (4 * ehalf, 4 * ehalf + 4)
                nc.sync.dma_start(
                    dst[:, c : c + 1, e0:e1, :],
                    bank_d[cot, ct, c : c + 1, e0:e1, :, :].rearrange(
                        "c e ci f -> ci c e f"
                    ),
                )

            def dma_x_band(b, band):
                r0, r1 = X_BANDS[band]
                for t, td in ((x8[b], x8_d), (x8r[b], x8r_d)):
                    v = t.rearrange("p (c h w) -> p c h w", c=CT, h=HP)
                    nc.sync.dma_start(
                        v[:, :, r0:r1, :],
                        td[b].rearrange("p (c h w) -> p c h w", c=CT, h=HP)[:, :, r0:r1, :],
                    )

            dma_bank(0, 0, 0, 0)
            dma_bank(0, 0, 0, 1)
            dma_x_band(0, 0)
            dma_bank(1, 0, 0, 0)
            dma_bank(1, 0, 0, 1)
            dma_x_band(0, 1)
            dma_bank(0, 0, 1)
            dma_bank(1, 0, 1)
            dma_x_band(0, 2)
            dma_bank(0, 0, 2)
            dma_bank(1, 0, 2)
            dma_x_band(0, 3)
            for t, td in ((x8[1], x8_d), (x8r[1], x8r_d)):
                nc.sync.dma_start(t[:], td[1])
            for c in range(NCH):
                dma_bank(0, 1, c)
                dma_bank(1, 1, c)

            # ---- engine-op builders ----
            def pe_combine(ct, c):
                """diag(r) combine of (b0, cot0, chunk c) on the PE; Act makes
                w8 straight from PSUM, DVE makes w8r = psum - w8."""
                ps = cpsump.tile([128, CCH], F32, tag="cps", name="cps")
                bv = bkv(ct, 0)
                for e in range(E):
                    nc.tensor.matmul(
                        ps[:],
                        sid[:, e * 128 : (e + 1) * 128],
                        bv[:, c, e, :],
                        start=(e == 0),
                        stop=(e == E - 1),
                    )
                psv = ps.rearrange("p (k o) -> p k o", k=CHK)
                k0 = CHK * c
                nc.scalar.copy(w8v(w8[(0, 0)])[:, k0 : k0 + CHK, ct, :], psv[:])
                nc.vector.tensor_tensor(
                    w8v(w8r[(0, 0)])[:, k0 : k0 + CHK, ct, :],
                    psv[:],
                    w8v(w8[(0, 0)])[:, k0 : k0 + CHK, ct, :],
                    Alu.subtract,
                )

            def dve_muls(b, ct, cot, c0, c1):
                """s_e = r_be * bank_e for taps [3*c0, 3*c1) (DVE, 4x bf16)."""
                bv = bkv(ct, cot)
                for e in range(E):
                    nc.vector.tensor_scalar_mul(
                        s[e][:, CCH * c0 : CCH * c1],
                        bv[:, c0:c1, e, :],
                        routf[:, b * E + e : b * E + e + 1],
                    )

            def dve_adds(ct):
                """7-add tree over s_0..s_7 into vres[ct] (DVE, 2x bf16)."""
                for i in range(4):
                    nc.vector.tensor_tensor(t4[i][:], s[2 * i][:], s[2 * i + 1][:], Alu.add)
                for i in range(2):
                    nc.vector.tensor_tensor(u2[i][:], t4[2 * i][:], t4[2 * i + 1][:], Alu.add)
                nc.vector.tensor_tensor(vres[ct][:], u2[0][:], u2[1][:], Alu.add)

            def act_w8_split(b, ct, cot):
                nc.scalar.copy(
                    w8v(w8[(b, cot)])[:, :, ct, :],
                    vres[ct].rearrange("p (k o) -> p k o", k=KK),
                )

            def dve_w8r_split(b, ct, cot):
                nc.vector.tensor_tensor(
                    w8v(w8r[(b, cot)])[:, :, ct, :],
                    vres[ct].rearrange("p (k o) -> p k o", k=KK),
                    w8v(w8[(b, cot)])[:, :, ct, :],
                    Alu.subtract,
                )

            def dve_unit(b, ct, cot):
                """Full combine + splits of one (b, ct, cot) unit."""
                dve_muls(b, ct, cot, 0, NCH)
                dve_adds(ct)
                act_w8_split(b, ct, cot)
                dve_w8r_split(b, ct, cot)

            def mm(pc, b, cot, p, kk, term, start, stop, rows=PIX_ROWS, h0=None):
                kh, kw = kk // KW, kk % KW
                wt = w8[(b, cot)] if term < 2 else w8r[(b, cot)]
                xt = x8[b] if term != 1 else x8r[b]
                xv = xt.rearrange("p (c h w) -> p c h w", c=CT, h=HP)
                if h0 is None:
                    h0 = p * PIX_ROWS
                nc.tensor.matmul(
                    pc[:],
                    w8v(wt)[:, kk, :, :],
                    xv[:, :, h0 + kh : h0 + kh + rows, kw : kw + W],
                    start=start,
                    stop=stop,
                    perf_mode=DR,
                    skip_group_check=True,
                )

            def evict(pc, b, cot, p, rows=PIX_ROWS, h0=None):
                if h0 is None:
                    h0 = p * PIX_ROWS
                ot = outsp.tile([128, rows * W], F32, tag="outs", name="outs")
                nc.scalar.mul(ot[:], pc[:], 1.0 / W_SCALE)
                nc.scalar.dma_start(
                    out_d[b, cot * 128 : (cot + 1) * 128, h0 : h0 + rows, :],
                    ot.rearrange("p (h w) -> p h w", h=rows),
                )

            def conv_pgroup(b, cot, p):
                pc = psump.tile([128, PIX_ROWS * W], F32, tag="ps", name="ps")
                first = True
                for c in range(NCH):
                    for term in range(3):
                        for kk in range(CHK * c, CHK * (c + 1)):
                            last = c == NCH - 1 and term == 2 and kk == KK - 1
                            mm(pc, b, cot, p, kk, term, start=first, stop=last)
                            first = False
                evict(pc, b, cot, p)

            # ---- warmup: 6 garbage matmuls ramp the PE p-state ----
            for i in range(6):
                wps = cpsump.tile([128, CCH], F32, tag="cps", name="cps")
                nc.tensor.matmul(
                    wps[:],
                    wtile[:, 0:128],
                    wtile[:, 0:CCH],
                    start=True,
                    stop=True,
                )

            # ---- (b0, cot0): chunk-passes over pixel groups 0-3 ----
            # DVE emission interleaved so b1's products run as banks land.
            dve_muls(1, 0, 0, 0, 1)
            pcs = {
                p: psump.tile([128, PIX_ROWS * W], F32, tag="ps", name="ps")
                for p in range(NP_PASS)
            }
            for c in range(NCH):
                pe_combine(0, c)
                pe_combine(1, c)
                for p in range(NP_PASS):
                    for term in range(3):
                        for kk in range(CHK * c, CHK * (c + 1)):
                            mm(
                                pcs[p], 0, 0, p, kk, term,
                                start=(c == 0 and term == 0 and kk == 0),
                                stop=(c == NCH - 1 and term == 2 and kk == KK - 1),
                            )
                if c < NCH - 1:
                    dve_muls(1, 0, 0, c + 1, c + 2)
            for p in range(NP_PASS):
                evict(pcs[p], 0, 0, p)

            # b1/ct0/cot0 combine finish (Act w8 after the p0-3 evictions)
            dve_adds(0)
            act_w8_split(1, 0, 0)
            dve_w8r_split(1, 0, 0)

            for p in range(NP_PASS, NPG):
                conv_pgroup(0, 0, p)

            dve_unit(1, 1, 0)

            for p in range(NP_PASS):
                conv_pgroup(1, 0, p)
            dve_unit(0, 0, 1)
            for p in range(NP_PASS, NPG):
                conv_pgroup(1, 0, p)
            dve_unit(0, 1, 1)

            for p in range(NP_PASS):
                conv_pgroup(0, 1, p)
            dve_unit(1, 0, 1)
            for p in range(NP_PASS, NPG):
                conv_pgroup(0, 1, p)
            dve_unit(1, 1, 1)

            # last group: final pixel group split in two 4-row halves to
            # shorten the eviction/DMA tail after the last matmul
            for p in range(NPG - 1):
                conv_pgroup(1, 1, p)
            for half in range(2):
                h0 = (NPG - 1) * PIX_ROWS + 4 * half
                pc = psump.tile([128, 4 * W], F32, tag="ps", name="ps")
                first = True
                for c in range(NCH):
                    for term in range(3):
                        for kk in range(CHK * c, CHK * (c + 1)):
                            last = c == NCH - 1 and term == 2 and kk == KK - 1
                            mm(pc, 1, 1, NPG - 1, kk, term, start=first, stop=last,
                               rows=4, h0=h0)
                            first = False
                evict(pc, 1, 1, NPG - 1, rows=4, h0=h0)
    nc.compile()
    return nc


def kernel(x, routing_weights, expert_weight):
    global LAST_RESULTS
    x = np.asarray(x, dtype=np.float32)
    r = np.asarray(routing_weights, dtype=np.float32)
    bank = np.asarray(expert_weight, dtype=np.float32)
    fp8 = ml_dtypes.float8_e4m3

    # x: [B, 256, H, W] -> padded per-ct layout [B, 128, ct, HP, WP],
    # split into fp8 value + fp8 residual (host-side quantization).
    xp = np.zeros((B, 128, CT, HP, WP), dtype=np.float32)
    xp[:, :, :, 1 : H + 1, 1 : W + 1] = x.reshape(B, CT, 128, H, W).transpose(0, 2, 1, 3, 4)
    x8 = xp.astype(fp8)
    x8r = (xp - x8.astype(np.float32)).astype(fp8)
    x8 = x8.reshape(B, 128, CT * HP * WP)
    x8r = x8r.reshape(B, 128, CT * HP * WP)

    # bank: [E, co*ci*kh*kw] -> [cot, ct, chunk, e, ci, (3 taps, co)] bf16
    bk = bank.reshape(E, CO_T, 128, CT, 128, NCH, CHK) * W_SCALE
    bank_t = np.ascontiguousarray(
        bk.transpose(1, 3, 5, 0, 4, 6, 2).reshape(CO_T, CT, NCH, E, 128, CCH)
    ).astype(ml_dtypes.bfloat16)

    if not _NC_CACHE:
        _NC_CACHE.append(_build())
    nc = _NC_CACHE[0]

    in_maps = []
    for c in range(N_CORES):
        rows = r[c * BPC : (c + 1) * BPC].reshape(BPC * E)
        rbc = np.ascontiguousarray(np.broadcast_to(rows[None, :], (128, BPC * E)))
        in_maps.append(
            {
                "x8": np.ascontiguousarray(x8[c * BPC : (c + 1) * BPC]),
                "x8r": np.ascontiguousarray(x8r[c * BPC : (c + 1) * BPC]),
                "bank": bank_t,
                "routb": rbc.astype(ml_dtypes.bfloat16),
                "routf": rbc,
            }
        )

    trace = bool(os.environ.get("KERNEL_TRACE"))
    try:
        res = run_bass_kernel_spmd(
            nc, in_maps, core_ids=list(range(N_CORES)), trace=trace
        )
    except ModuleNotFoundError:
        if not trace:
            raise
        # Tracing unavailable in this environment (no axon NTFF hook).
        res = run_bass_kernel_spmd(
            nc, in_maps, core_ids=list(range(N_CORES)), trace=False
        )
    LAST_RESULTS = res
    return np.concatenate([rr["out"] for rr in res.results], axis=0)


# revision 11
# speedup vs baseline: 1.4362x; 1.0128x over previous
"""CondConv2d Trainium2 kernel — fp8 DoubleRow implicit GEMM.

Per-sample expert-combined 3x3 conv (B=16, 256->256 ch, 64x64, fp32),
data-parallel over batch on 8 NeuronCores (2 samples/core).

Math: out_b = conv(x_b, W_b), W_b = sum_e r_be Bank_e. All MACs run on the
PE in fp8e4m3 DoubleRow mode (2 K-tiles per matmul, 2 cols/cycle) with a
3-term error-compensated decomposition:

    x = x8 + x8r     (fp8 value + fp8 residual, host-side quantization)
    W*S = w8 + w8r   (fp8 split computed on device; S=512 lifts the tiny
                      combined weights out of fp8's subnormal range, and
                      the eviction copy descales by 1/S — exact, power of 2)
    out ~= (x8*w8 + x8r*w8 + x8*w8r) / S    (dropped term x8r*w8r ~ 0.1%)

Per 8-row pixel group: 27 DoubleRow matmuls (9 taps x 3 terms, each
contracting both 128-channel tiles at once), N=512 at 0.5 cycles/row.

Device schedule per core (engines pipelined, all queues in-order):
  - PE: 6 garbage warmup matmuls (p-state ramp) -> diag(r) combine of
    (b0, cot0) one 3-tap chunk at a time, each chunk followed by a conv
    pass over pixel groups 0-3 consuming just that chunk -> single-pass
    pixel groups 4-7 -> remaining three (b, cot) groups single-pass.
  - DVE: b1/cot0 products chunk-wise as bank chunks land, w8r splits for
    the PE-combined chunks, then full-unit combines (8 muls in 4x bf16
    mode + 7-add tree in 2x mode) for the remaining (b, ct, cot) units.
  - Act: w8 fp8 splits + PSUM evictions (with 1/S descale) + output DMAs,
    emission-ordered so no split head-blocks an eviction.
  - x arrives pre-padded/pre-split from the host as fp8 [ci128, ct2, 66, 66]
    in 4 row-bands for sample 0; no on-device padding or scatter work.
  - Last pixel group split in two 4-row halves to shorten the drain tail.
"""

import os

import numpy as np
import ml_dtypes

import concourse.tile as tile
from concourse import bacc, mybir
from concourse.bass_utils import run_bass_kernel_spmd

B, C_IN, C_OUT, H, W = 16, 256, 256, 64, 64
KH = KW = 3
KK = KH * KW
E = 8
N_CORES = 8
BPC = B // N_CORES  # samples per core

HP, WP = H + 2, W + 2  # zero-padded image dims
CT = 2  # ci tiles (DoubleRow K-tile pairs)
CO_T = 2  # co tiles (output-channel halves)
NCH = 3  # kk chunks
CHK = 3  # taps per chunk
CCH = CHK * 128  # chunk free size in weight layout
PIX_ROWS = 8  # image rows per conv matmul -> N = 8*64 = 512
NPG = H // PIX_ROWS  # pixel groups per (b, cot)
NP_PASS = 4  # pixel groups covered by the chunk-pass window

# Combined weights are ~1e-2 — inside fp8e4m3's subnormal range, where the
# residual split can't resolve anything. Scale the bank into the normal
# range host-side and descale in the eviction copy (power of two: exact).
W_SCALE = 512.0

F32 = mybir.dt.float32
BF16 = mybir.dt.bfloat16
F8 = mybir.dt.float8e4
Alu = mybir.AluOpType
DR = mybir.MatmulPerfMode.DoubleRow

X_BANDS = [(0, 18), (18, 34), (34, 50), (50, 66)]  # padded-row bands, sample 0

LAST_RESULTS = None  # stashed BassKernelResults for test harness introspection
_NC_CACHE = []


def _build():
    nc = bacc.Bacc("TRN2", target_bir_lowering=False, debug=False, enable_asserts=False)
    x8_d = nc.dram_tensor("x8", [BPC, 128, CT * HP * WP], F8, kind="ExternalInput")
    x8r_d = nc.dram_tensor("x8r", [BPC, 128, CT * HP * WP], F8, kind="ExternalInput")
    # [cot, ct, chunk, e, ci, chunk-taps*co]
    bank_d = nc.dram_tensor("bank", [CO_T, CT, NCH, E, 128, CCH], BF16, kind="ExternalInput")
    routb_d = nc.dram_tensor("routb", [128, BPC * E], BF16, kind="ExternalInput")
    routf_d = nc.dram_tensor("routf", [128, BPC * E], F32, kind="ExternalInput")
    out_d = nc.dram_tensor("out", [BPC, C_OUT, H, W], F32, kind="ExternalOutput")

    with tile.TileContext(nc) as tc:
        with (
            tc.tile_pool(name="const", bufs=1) as constp,
            tc.tile_pool(name="xp", bufs=1) as xpp,
            tc.tile_pool(name="bank", bufs=1) as bankp,
            tc.tile_pool(name="w8", bufs=1) as w8p,
            tc.tile_pool(name="scr", bufs=1) as scrp,
            tc.tile_pool(name="outs", bufs=6) as outsp,
            tc.tile_pool(name="cpsum", bufs=2, space="PSUM") as cpsump,
            tc.tile_pool(name="psum", bufs=6, space="PSUM") as psump,
        ):
            routb = constp.tile([128, BPC * E], BF16, tag="routb")
            routf = constp.tile([128, BPC * E], F32, tag="routf")
            nc.sync.dma_start(routb[:], routb_d[:])
            nc.sync.dma_start(routf[:], routf_d[:])

            # warmup scratch (zeros) for p-state ramp matmuls
            wtile = constp.tile([128, 512], F8, tag="warm")
            nc.gpsimd.memset(wtile[:], 0)

            # diag(r_b0,e) tiles for the PE-side startup combine (sample 0)
            sid = constp.tile([128, E * 128], BF16, tag="sid")
            for e in range(E):
                nc.gpsimd.affine_select(
                    sid[:, e * 128 : (e + 1) * 128],
                    routb[:, e : e + 1].broadcast_to((128, 128)),
                    pattern=[[1, 128]],
                    compare_op=Alu.is_equal,
                    fill=0.0,
                    base=0,
                    channel_multiplier=-1,
                )

            # ---- SBUF tiles ----
            x8 = {}
            x8r = {}
            for b in range(BPC):
                x8[b] = xpp.tile([128, CT * HP * WP], F8, tag=f"x8_{b}", name=f"x8_{b}")
                x8r[b] = xpp.tile([128, CT * HP * WP], F8, tag=f"x8r_{b}", name=f"x8r_{b}")

            bkt = {}
            for cot in range(CO_T):
                for ct in range(CT):
                    bkt[(ct, cot)] = bankp.tile(
                        [128, NCH * E * CCH], BF16, tag=f"bk{ct}{cot}", name=f"bk{ct}{cot}"
                    )

            def bkv(ct, cot):
                return bkt[(ct, cot)].rearrange("p (c e f) -> p c e f", c=NCH, e=E)

            w8 = {}
            w8r = {}
            for b in range(BPC):
                for cot in range(CO_T):
                    w8[(b, cot)] = w8p.tile(
                        [128, KK * CT * 128], F8, tag=f"w8_{b}{cot}", name=f"w8_{b}{cot}"
                    )
                    w8r[(b, cot)] = w8p.tile(
                        [128, KK * CT * 128], F8, tag=f"w8r_{b}{cot}", name=f"w8r_{b}{cot}"
                    )

            def w8v(t):
                return t.rearrange("p (k c o) -> p k c o", k=KK, c=CT)

            # DVE combine scratch: per-expert products + add tree + results
            s = {e: scrp.tile([128, KK * 128], BF16, tag=f"s{e}", name=f"s{e}") for e in range(E)}
            t4 = {i: scrp.tile([128, KK * 128], BF16, tag=f"t{i}", name=f"t{i}") for i in range(4)}
            u2 = {i: scrp.tile([128, KK * 128], BF16, tag=f"u{i}", name=f"u{i}") for i in range(2)}
            vres = {c: scrp.tile([128, KK * 128], BF16, tag=f"v{c}", name=f"v{c}") for c in range(CT)}

            # ---- input DMA stream (SP queue, priority order) ----
            def dma_bank(ct, cot, c, ehalf=None):
                dst = bkv(ct, cot)
                e0, e1 = (0, E) if ehalf is None else (4 * ehalf, 4 * ehalf + 4)
                nc.sync.dma_start(
                    dst[:, c : c + 1, e0:e1, :],
                    bank_d[cot, ct, c : c + 1, e0:e1, :, :].rearrange(
                        "c e ci f -> ci c e f"
                    ),
                )

            def dma_x_band(b, band):
                r0, r1 = X_BANDS[band]
                for t, td in ((x8[b], x8_d), (x8r[b], x8r_d)):
                    v = t.rearrange("p (c h w) -> p c h w", c=CT, h=HP)
                    nc.sync.dma_start(
                        v[:, :, r0:r1, :],
                        td[b].rearrange("p (c h w) -> p c h w", c=CT, h=HP)[:, :, r0:r1, :],
                    )

            dma_bank(0, 0, 0, 0)
            dma_bank(0, 0, 0, 1)
            dma_x_band(0, 0)
            dma_bank(1, 0, 0, 0)
            dma_bank(1, 0, 0, 1)
            dma_bank(0, 0, 1)
            dma_x_band(0, 1)
            dma_bank(1, 0, 1)
            dma_x_band(0, 2)
            dma_bank(0, 0, 2)
            dma_bank(1, 0, 2)
            dma_x_band(0, 3)
            for t, td in ((x8[1], x8_d), (x8r[1], x8r_d)):
                nc.sync.dma_start(t[:], td[1])
            for c in range(NCH):
                dma_bank(0, 1, c)
                dma_bank(1, 1, c)

            # ---- engine-op builders ----
            def pe_combine(ct, c):
                """diag(r) combine of (b0, cot0, chunk c) on the PE; Act makes
                w8 straight from PSUM, DVE makes w8r = psum - w8."""
                ps = cpsump.tile([128, CCH], F32, tag="cps", name="cps")
                bv = bkv(ct, 0)
                for e in range(E):
                    nc.tensor.matmul(
                        ps[:],
                        sid[:, e * 128 : (e + 1) * 128],
                        bv[:, c, e, :],
                        start=(e == 0),
                        stop=(e == E - 1),
                    )
                psv = ps.rearrange("p (k o) -> p k o", k=CHK)
                k0 = CHK * c
                nc.scalar.copy(w8v(w8[(0, 0)])[:, k0 : k0 + CHK, ct, :], psv[:])
                nc.vector.tensor_tensor(
                    w8v(w8r[(0, 0)])[:, k0 : k0 + CHK, ct, :],
                    psv[:],
                    w8v(w8[(0, 0)])[:, k0 : k0 + CHK, ct, :],
                    Alu.subtract,
                )

            def dve_muls(b, ct, cot, c0, c1):
                """s_e = r_be * bank_e for taps [3*c0, 3*c1) (DVE, 4x bf16)."""
                bv = bkv(ct, cot)
                for e in range(E):
                    nc.vector.tensor_scalar_mul(
                        s[e][:, CCH * c0 : CCH * c1],
                        bv[:, c0:c1, e, :],
                        routf[:, b * E + e : b * E + e + 1],
                    )

            def dve_adds(ct):
                """7-add tree over s_0..s_7 into vres[ct] (DVE, 2x bf16)."""
                for i in range(4):
                    nc.vector.tensor_tensor(t4[i][:], s[2 * i][:], s[2 * i + 1][:], Alu.add)
                for i in range(2):
                    nc.vector.tensor_tensor(u2[i][:], t4[2 * i][:], t4[2 * i + 1][:], Alu.add)
                nc.vector.tensor_tensor(vres[ct][:], u2[0][:], u2[1][:], Alu.add)

            def act_w8_split(b, ct, cot):
                nc.scalar.copy(
                    w8v(w8[(b, cot)])[:, :, ct, :],
                    vres[ct].rearrange("p (k o) -> p k o", k=KK),
                )

            def dve_w8r_split(b, ct, cot):
                nc.vector.tensor_tensor(
                    w8v(w8r[(b, cot)])[:, :, ct, :],
                    vres[ct].rearrange("p (k o) -> p k o", k=KK),
                    w8v(w8[(b, cot)])[:, :, ct, :],
                    Alu.subtract,
                )

            def dve_unit(b, ct, cot):
                """Full combine + splits of one (b, ct, cot) unit."""
                dve_muls(b, ct, cot, 0, NCH)
                dve_adds(ct)
                act_w8_split(b, ct, cot)
                dve_w8r_split(b, ct, cot)

            def dve_chunk_stt(b, ct, cot, c):
                """One 3-tap chunk of (b, ct, cot) via an FMA chain on the
                DVE (no scratch tiles; used where s/t4/u2 are busy)."""
                bv = bkv(ct, cot)
                k0 = CHK * c
                vsl = vres[ct][:, k0 * 128 : (k0 + CHK) * 128]
                nc.vector.tensor_scalar_mul(
                    vsl, bv[:, c, 0, :], routf[:, b * E : b * E + 1]
                )
                for e in range(1, E):
                    nc.vector.scalar_tensor_tensor(
                        vsl, bv[:, c, e, :],
                        routf[:, b * E + e : b * E + e + 1],
                        vsl, Alu.mult, Alu.add,
                    )
                vv = vsl.rearrange("p (k o) -> p k o", k=CHK)
                nc.scalar.copy(w8v(w8[(b, cot)])[:, k0 : k0 + CHK, ct, :], vv[:])
                nc.vector.tensor_tensor(
                    w8v(w8r[(b, cot)])[:, k0 : k0 + CHK, ct, :],
                    vv[:],
                    w8v(w8[(b, cot)])[:, k0 : k0 + CHK, ct, :],
                    Alu.subtract,
                )

            def mm(pc, b, cot, p, kk, term, start, stop, rows=PIX_ROWS, h0=None):
                kh, kw = kk // KW, kk % KW
                wt = w8[(b, cot)] if term < 2 else w8r[(b, cot)]
                xt = x8[b] if term != 1 else x8r[b]
                xv = xt.rearrange("p (c h w) -> p c h w", c=CT, h=HP)
                if h0 is None:
                    h0 = p * PIX_ROWS
                nc.tensor.matmul(
                    pc[:],
                    w8v(wt)[:, kk, :, :],
                    xv[:, :, h0 + kh : h0 + kh + rows, kw : kw + W],
                    start=start,
                    stop=stop,
                    perf_mode=DR,
                    skip_group_check=True,
                )

            def evict(pc, b, cot, p, rows=PIX_ROWS, h0=None):
                if h0 is None:
                    h0 = p * PIX_ROWS
                ot = outsp.tile([128, rows * W], F32, tag="outs", name="outs")
                nc.scalar.mul(ot[:], pc[:], 1.0 / W_SCALE)
                nc.sync.dma_start(
                    out_d[b, cot * 128 : (cot + 1) * 128, h0 : h0 + rows, :],
                    ot.rearrange("p (h w) -> p h w", h=rows),
                )

            def conv_pgroup(b, cot, p):
                pc = psump.tile([128, PIX_ROWS * W], F32, tag="ps", name="ps")
                first = True
                for c in range(NCH):
                    for term in range(3):
                        for kk in range(CHK * c, CHK * (c + 1)):
                            last = c == NCH - 1 and term == 2 and kk == KK - 1
                            mm(pc, b, cot, p, kk, term, start=first, stop=last)
                            first = False
                evict(pc, b, cot, p)

            # ---- warmup: 6 garbage matmuls ramp the PE p-state ----
            for i in range(6):
                wps = cpsump.tile([128, CCH], F32, tag="cps", name="cps")
                nc.tensor.matmul(
                    wps[:],
                    wtile[:, 0:128],
                    wtile[:, 0:CCH],
                    start=True,
                    stop=True,
                )

            # ---- (b0, cot0): chunk-passes over pixel groups 0-3 ----
            # PE combines chunks c0 (both ct), c1/ct1, c2 (both ct); the DVE
            # takes c1/ct0 via an FMA chain plus b1's products as banks land.
            def cpass(pcs, c, p_range):
                for p in p_range:
                    for term in range(3):
                        for kk in range(CHK * c, CHK * (c + 1)):
                            mm(
                                pcs[p], 0, 0, p, kk, term,
                                start=(c == 0 and term == 0 and kk == 0),
                                stop=(c == NCH - 1 and term == 2 and kk == KK - 1),
                            )

            dve_muls(1, 0, 0, 0, 1)
            pcs = {
                p: psump.tile([128, PIX_ROWS * W], F32, tag="ps", name="ps")
                for p in range(NP_PASS)
            }
            pe_combine(0, 0)
            pe_combine(1, 0)
            cpass(pcs, 0, range(0, 2))
            dve_chunk_stt(0, 0, 0, 1)  # c1/ct0 on the DVE
            cpass(pcs, 0, range(2, NP_PASS))
            pe_combine(1, 1)  # c1/ct1
            dve_muls(1, 0, 0, 1, 2)
            cpass(pcs, 1, range(0, NP_PASS))
            dve_muls(1, 0, 0, 2, 3)
            pe_combine(0, 2)
            pe_combine(1, 2)
            cpass(pcs, 2, range(0, NP_PASS))
            for p in range(NP_PASS):
                evict(pcs[p], 0, 0, p)

            # b1/cot0 combine finish, interleaved into pixel groups 4-7
            # so both w8(b1) splits land before the (b1, cot0) group starts
            dve_adds(0)
            dve_muls(1, 1, 0, 0, NCH)
            act_w8_split(1, 0, 0)
            dve_w8r_split(1, 0, 0)
            conv_pgroup(0, 0, 4)
            conv_pgroup(0, 0, 5)
            dve_adds(1)
            act_w8_split(1, 1, 0)
            dve_w8r_split(1, 1, 0)
            conv_pgroup(0, 0, 6)
            conv_pgroup(0, 0, 7)

            for p in range(NP_PASS):
                conv_pgroup(1, 0, p)
            dve_unit(0, 0, 1)
            for p in range(NP_PASS, NPG):
                conv_pgroup(1, 0, p)
            dve_unit(0, 1, 1)

            for p in range(NP_PASS):
                conv_pgroup(0, 1, p)
            dve_unit(1, 0, 1)
            for p in range(NP_PASS, NPG):
                conv_pgroup(0, 1, p)
            dve_unit(1, 1, 1)

            # last group: final pixel group split in two 4-row halves to
            # shorten the eviction/DMA tail after the last matmul
            for p in range(NPG - 1):
                conv_pgroup(1, 1, p)
            for half in range(2):
                h0 = (NPG - 1) * PIX_ROWS + 4 * half
                pc = psump.tile([128, 4 * W], F32, tag="ps", name="ps")
                first = True
                for c in range(NCH):
                    for term in range(3):
                        for kk in range(CHK * c, CHK * (c + 1)):
                            last = c == NCH - 1 and term == 2 and kk == KK - 1
                            mm(pc, 1, 1, NPG - 1, kk, term, start=first, stop=last,
                               rows=4, h0=h0)
                            first = False
                evict(pc, 1, 1, NPG - 1, rows=4, h0=h0)
    nc.compile()
    return nc


def kernel(x, routing_weights, expert_weight):
    global LAST_RESULTS
    x = np.asarray(x, dtype=np.float32)
    r = np.asarray(routing_weights, dtype=np.float32)
    bank = np.asarray(expert_weight, dtype=np.float32)
    fp8 = ml_dtypes.float8_e4m3

    # x: [B, 256, H, W] -> padded per-ct layout [B, 128, ct, HP, WP],
    # split into fp8 value + fp8 residual (host-side quantization).
    xp = np.zeros((B, 128, CT, HP, WP), dtype=np.float32)
    xp[:, :, :, 1 : H + 1, 1 : W + 1] = x.reshape(B, CT, 128, H, W).transpose(0, 2, 1, 3, 4)
    x8 = xp.astype(fp8)
    x8r = (xp - x8.astype(np.float32)).astype(fp8)
    x8 = x8.reshape(B, 128, CT * HP * WP)
    x8r = x8r.reshape(B, 128, CT * HP * WP)

    # bank: [E, co*ci*kh*kw] -> [cot, ct, chunk, e, ci, (3 taps, co)] bf16
    bk = bank.reshape(E, CO_T, 128, CT, 128, NCH, CHK) * W_SCALE
    bank_t = np.ascontiguousarray(
        bk.transpose(1, 3, 5, 0, 4, 6, 2).reshape(CO_T, CT, NCH, E, 128, CCH)
    ).astype(ml_dtypes.bfloat16)

    if not _NC_CACHE:
        _NC_CACHE.append(_build())
    nc = _NC_CACHE[0]

    in_maps = []
    for c in range(N_CORES):
        rows = r[c * BPC : (c + 1) * BPC].reshape(BPC * E)
        rbc = np.ascontiguousarray(np.broadcast_to(rows[None, :], (128, BPC * E)))
        in_maps.append(
            {
                "x8": np.ascontiguousarray(x8[c * BPC : (c + 1) * BPC]),
                "x8r": np.ascontiguousarray(x8r[c * BPC : (c + 1) * BPC]),
                "bank": bank_t,
                "routb": rbc.astype(ml_dtypes.bfloat16),
                "routf": rbc,
            }
        )

    trace = bool(os.environ.get("KERNEL_TRACE"))
    try:
        res = run_bass_kernel_spmd(
            nc, in_maps, core_ids=list(range(N_CORES)), trace=trace
        )
    except ModuleNotFoundError:
        if not trace:
            raise
        # Tracing unavailable in this environment (no axon NTFF hook).
        res = run_bass_kernel_spmd(
            nc, in_maps, core_ids=list(range(N_CORES)), trace=False
        )
    LAST_RESULTS = res
    return np.concatenate([rr["out"] for rr in res.results], axis=0)


# revision 12
# speedup vs baseline: 1.4416x; 1.0038x over previous
"""CondConv2d Trainium2 kernel — fp8 DoubleRow implicit GEMM.

Per-sample expert-combined 3x3 conv (B=16, 256->256 ch, 64x64, fp32),
data-parallel over batch on 8 NeuronCores (2 samples/core).

Math: out_b = conv(x_b, W_b), W_b = sum_e r_be Bank_e. All MACs run on the
PE in fp8e4m3 DoubleRow mode (2 K-tiles per matmul, 2 cols/cycle) with a
3-term error-compensated decomposition:

    x = x8 + x8r     (fp8 value + fp8 residual, host-side quantization)
    W*S = w8 + w8r   (fp8 split computed on device; S=512 lifts the tiny
                      combined weights out of fp8's subnormal range, and
                      the eviction copy descales by 1/S — exact, power of 2)
    out ~= (x8*w8 + x8r*w8 + x8*w8r) / S    (dropped term x8r*w8r ~ 0.1%)

Per 8-row pixel group: 27 DoubleRow matmuls (9 taps x 3 terms, each
contracting both 128-channel tiles at once), N=512 at 0.5 cycles/row.

Device schedule per core (engines pipelined, all queues in-order):
  - PE: 6 garbage warmup matmuls (p-state ramp) -> diag(r) combine of
    (b0, cot0) one 3-tap chunk at a time, each chunk followed by a conv
    pass over pixel groups 0-3 consuming just that chunk -> single-pass
    pixel groups 4-7 -> remaining three (b, cot) groups single-pass.
  - DVE: b1/cot0 products chunk-wise as bank chunks land, w8r splits for
    the PE-combined chunks, then full-unit combines (8 muls in 4x bf16
    mode + 7-add tree in 2x mode) for the remaining (b, ct, cot) units.
  - Act: w8 fp8 splits + PSUM evictions (with 1/S descale) + output DMAs,
    emission-ordered so no split head-blocks an eviction.
  - x arrives pre-padded/pre-split from the host as fp8 [ci128, ct2, 66, 66]
    in 4 row-bands for sample 0; no on-device padding or scatter work.
  - Last pixel group split in two 4-row halves to shorten the drain tail.
"""

import os

import numpy as np
import ml_dtypes

import concourse.tile as tile
from concourse import bacc, mybir
from concourse.bass_utils import run_bass_kernel_spmd

B, C_IN, C_OUT, H, W = 16, 256, 256, 64, 64
KH = KW = 3
KK = KH * KW
E = 8
N_CORES = 8
BPC = B // N_CORES  # samples per core

HP, WP = H + 2, W + 2  # zero-padded image dims
CT = 2  # ci tiles (DoubleRow K-tile pairs)
CO_T = 2  # co tiles (output-channel halves)
NCH = 3  # kk chunks
CHK = 3  # taps per chunk
CCH = CHK * 128  # chunk free size in weight layout
PIX_ROWS = 8  # image rows per conv matmul -> N = 8*64 = 512
NPG = H // PIX_ROWS  # pixel groups per (b, cot)
NP_PASS = 4  # pixel groups covered by the chunk-pass window

# Combined weights are ~1e-2 — inside fp8e4m3's subnormal range, where the
# residual split can't resolve anything. Scale the bank into the normal
# range host-side and descale in the eviction copy (power of two: exact).
W_SCALE = 512.0

F32 = mybir.dt.float32
BF16 = mybir.dt.bfloat16
F8 = mybir.dt.float8e4
Alu = mybir.AluOpType
DR = mybir.MatmulPerfMode.DoubleRow

X_BANDS = [(0, 18), (18, 34), (34, 50), (50, 66)]  # padded-row bands, sample 0

LAST_RESULTS = None  # stashed BassKernelResults for test harness introspection
_NC_CACHE = []


def _build():
    nc = bacc.Bacc("TRN2", target_bir_lowering=False, debug=False, enable_asserts=False)
    x8_d = nc.dram_tensor("x8", [BPC, 128, CT * HP * WP], F8, kind="ExternalInput")
    x8r_d = nc.dram_tensor("x8r", [BPC, 128, CT * HP * WP], F8, kind="ExternalInput")
    # [cot, ct, chunk, e, ci, chunk-taps*co]
    bank_d = nc.dram_tensor("bank", [CO_T, CT, NCH, E, 128, CCH], BF16, kind="ExternalInput")
    routb_d = nc.dram_tensor("routb", [128, BPC * E], BF16, kind="ExternalInput")
    out_d = nc.dram_tensor("out", [BPC, C_OUT, H, W], F32, kind="ExternalOutput")

    with tile.TileContext(nc) as tc:
        with (
            tc.tile_pool(name="const", bufs=1) as constp,
            tc.tile_pool(name="xp", bufs=1) as xpp,
            tc.tile_pool(name="bank", bufs=1) as bankp,
            tc.tile_pool(name="w8", bufs=1) as w8p,
            tc.tile_pool(name="scr", bufs=1) as scrp,
            tc.tile_pool(name="outs", bufs=6) as outsp,
            tc.tile_pool(name="cpsum", bufs=2, space="PSUM") as cpsump,
            tc.tile_pool(name="psum", bufs=6, space="PSUM") as psump,
        ):
            routb = constp.tile([128, BPC * E], BF16, tag="routb")
            routf = constp.tile([128, BPC * E], F32, tag="routf")
            nc.sync.dma_start(routb[:], routb_d[:])
            nc.vector.tensor_copy(routf[:], routb[:])

            # warmup scratch (zeros) for p-state ramp matmuls
            wtile = constp.tile([128, 512], F8, tag="warm")
            nc.gpsimd.memset(wtile[:], 0)

            # diag(r_b0,e) tiles for the PE-side startup combine (sample 0)
            sid = constp.tile([128, E * 128], BF16, tag="sid")
            for e in range(E):
                nc.gpsimd.affine_select(
                    sid[:, e * 128 : (e + 1) * 128],
                    routb[:, e : e + 1].broadcast_to((128, 128)),
                    pattern=[[1, 128]],
                    compare_op=Alu.is_equal,
                    fill=0.0,
                    base=0,
                    channel_multiplier=-1,
                )

            # ---- SBUF tiles ----
            x8 = {}
            x8r = {}
            for b in range(BPC):
                x8[b] = xpp.tile([128, CT * HP * WP], F8, tag=f"x8_{b}", name=f"x8_{b}")
                x8r[b] = xpp.tile([128, CT * HP * WP], F8, tag=f"x8r_{b}", name=f"x8r_{b}")

            bkt = {}
            for cot in range(CO_T):
                for ct in range(CT):
                    bkt[(ct, cot)] = bankp.tile(
                        [128, NCH * E * CCH], BF16, tag=f"bk{ct}{cot}", name=f"bk{ct}{cot}"
                    )

            def bkv(ct, cot):
                return bkt[(ct, cot)].rearrange("p (c e f) -> p c e f", c=NCH, e=E)

            w8 = {}
            w8r = {}
            for b in range(BPC):
                for cot in range(CO_T):
                    w8[(b, cot)] = w8p.tile(
                        [128, KK * CT * 128], F8, tag=f"w8_{b}{cot}", name=f"w8_{b}{cot}"
                    )
                    w8r[(b, cot)] = w8p.tile(
                        [128, KK * CT * 128], F8, tag=f"w8r_{b}{cot}", name=f"w8r_{b}{cot}"
                    )

            def w8v(t):
                return t.rearrange("p (k c o) -> p k c o", k=KK, c=CT)

            # DVE combine scratch: per-expert products + add tree + results
            s = {e: scrp.tile([128, KK * 128], BF16, tag=f"s{e}", name=f"s{e}") for e in range(E)}
            t4 = {i: scrp.tile([128, KK * 128], BF16, tag=f"t{i}", name=f"t{i}") for i in range(4)}
            u2 = {i: scrp.tile([128, KK * 128], BF16, tag=f"u{i}", name=f"u{i}") for i in range(2)}
            vres = {c: scrp.tile([128, KK * 128], BF16, tag=f"v{c}", name=f"v{c}") for c in range(CT)}

            # ---- input DMA stream (SP queue, priority order) ----
            def dma_bank(ct, cot, c, ehalf=None):
                dst = bkv(ct, cot)
                e0, e1 = (0, E) if ehalf is None else (4 * ehalf, 4 * ehalf + 4)
                nc.sync.dma_start(
                    dst[:, c : c + 1, e0:e1, :],
                    bank_d[cot, ct, c : c + 1, e0:e1, :, :].rearrange(
                        "c e ci f -> ci c e f"
                    ),
                )

            def dma_x_band(b, band):
                r0, r1 = X_BANDS[band]
                for t, td in ((x8[b], x8_d), (x8r[b], x8r_d)):
                    v = t.rearrange("p (c h w) -> p c h w", c=CT, h=HP)
                    nc.sync.dma_start(
                        v[:, :, r0:r1, :],
                        td[b].rearrange("p (c h w) -> p c h w", c=CT, h=HP)[:, :, r0:r1, :],
                    )

            dma_bank(0, 0, 0, 0)
            dma_bank(0, 0, 0, 1)
            dma_bank(1, 0, 0, 0)
            dma_bank(1, 0, 0, 1)
            dma_x_band(0, 0)
            dma_bank(0, 0, 1)
            dma_bank(1, 0, 1)
            dma_x_band(0, 1)
            dma_bank(0, 0, 2)
            dma_bank(1, 0, 2)
            dma_x_band(0, 2)
            dma_x_band(0, 3)
            for t, td in ((x8[1], x8_d), (x8r[1], x8r_d)):
                nc.sync.dma_start(t[:], td[1])
            for ct in range(CT):
                for c in range(NCH):
                    dma_bank(ct, 1, c)

            # ---- engine-op builders ----
            def pe_combine(ct, c):
                """diag(r) combine of (b0, cot0, chunk c) on the PE; Act makes
                w8 straight from PSUM, DVE makes w8r = psum - w8."""
                ps = cpsump.tile([128, CCH], F32, tag="cps", name="cps")
                bv = bkv(ct, 0)
                for e in range(E):
                    nc.tensor.matmul(
                        ps[:],
                        sid[:, e * 128 : (e + 1) * 128],
                        bv[:, c, e, :],
                        start=(e == 0),
                        stop=(e == E - 1),
                    )
                psv = ps.rearrange("p (k o) -> p k o", k=CHK)
                k0 = CHK * c
                nc.scalar.copy(w8v(w8[(0, 0)])[:, k0 : k0 + CHK, ct, :], psv[:])
                nc.vector.tensor_tensor(
                    w8v(w8r[(0, 0)])[:, k0 : k0 + CHK, ct, :],
                    psv[:],
                    w8v(w8[(0, 0)])[:, k0 : k0 + CHK, ct, :],
                    Alu.subtract,
                )

            def dve_muls(b, ct, cot, c0, c1):
                """s_e = r_be * bank_e for taps [3*c0, 3*c1) (DVE, 4x bf16)."""
                bv = bkv(ct, cot)
                for e in range(E):
                    nc.vector.tensor_scalar_mul(
                        s[e][:, CCH * c0 : CCH * c1],
                        bv[:, c0:c1, e, :],
                        routf[:, b * E + e : b * E + e + 1],
                    )

            def dve_adds(ct):
                """7-add tree over s_0..s_7 into vres[ct] (DVE, 2x bf16)."""
                for i in range(4):
                    nc.vector.tensor_tensor(t4[i][:], s[2 * i][:], s[2 * i + 1][:], Alu.add)
                for i in range(2):
                    nc.vector.tensor_tensor(u2[i][:], t4[2 * i][:], t4[2 * i + 1][:], Alu.add)
                nc.vector.tensor_tensor(vres[ct][:], u2[0][:], u2[1][:], Alu.add)

            def act_w8_split(b, ct, cot):
                nc.scalar.copy(
                    w8v(w8[(b, cot)])[:, :, ct, :],
                    vres[ct].rearrange("p (k o) -> p k o", k=KK),
                )

            def dve_w8r_split(b, ct, cot):
                nc.vector.tensor_tensor(
                    w8v(w8r[(b, cot)])[:, :, ct, :],
                    vres[ct].rearrange("p (k o) -> p k o", k=KK),
                    w8v(w8[(b, cot)])[:, :, ct, :],
                    Alu.subtract,
                )

            def dve_unit(b, ct, cot):
                """Full combine + splits of one (b, ct, cot) unit."""
                dve_muls(b, ct, cot, 0, NCH)
                dve_adds(ct)
                act_w8_split(b, ct, cot)
                dve_w8r_split(b, ct, cot)

            def dve_chunk_stt_chain(b, ct, cot, c):
                """One 3-tap chunk of (b, ct, cot) via an FMA chain on the
                DVE (no scratch tiles; used where s/t4/u2 are busy)."""
                bv = bkv(ct, cot)
                k0 = CHK * c
                vsl = vres[ct][:, k0 * 128 : (k0 + CHK) * 128]
                nc.vector.tensor_scalar_mul(
                    vsl, bv[:, c, 0, :], routf[:, b * E : b * E + 1]
                )
                for e in range(1, E):
                    nc.vector.scalar_tensor_tensor(
                        vsl, bv[:, c, e, :],
                        routf[:, b * E + e : b * E + e + 1],
                        vsl, Alu.mult, Alu.add,
                    )

            def dve_chunk_stt_splits(b, ct, cot, c):
                k0 = CHK * c
                vv = vres[ct][:, k0 * 128 : (k0 + CHK) * 128].rearrange(
                    "p (k o) -> p k o", k=CHK
                )
                nc.scalar.copy(w8v(w8[(b, cot)])[:, k0 : k0 + CHK, ct, :], vv[:])
                nc.vector.tensor_tensor(
                    w8v(w8r[(b, cot)])[:, k0 : k0 + CHK, ct, :],
                    vv[:],
                    w8v(w8[(b, cot)])[:, k0 : k0 + CHK, ct, :],
                    Alu.subtract,
                )

            def mm(pc, b, cot, p, kk, term, start, stop, rows=PIX_ROWS, h0=None):
                kh, kw = kk // KW, kk % KW
                wt = w8[(b, cot)] if term < 2 else w8r[(b, cot)]
                xt = x8[b] if term != 1 else x8r[b]
                xv = xt.rearrange("p (c h w) -> p c h w", c=CT, h=HP)
                if h0 is None:
                    h0 = p * PIX_ROWS
                nc.tensor.matmul(
                    pc[:],
                    w8v(wt)[:, kk, :, :],
                    xv[:, :, h0 + kh : h0 + kh + rows, kw : kw + W],
                    start=start,
                    stop=stop,
                    perf_mode=DR,
                    skip_group_check=True,
                )

            def evict(pc, b, cot, p, rows=PIX_ROWS, h0=None):
                if h0 is None:
                    h0 = p * PIX_ROWS
                ot = outsp.tile([128, rows * W], F32, tag="outs", name="outs")
                nc.scalar.mul(ot[:], pc[:], 1.0 / W_SCALE)
                nc.sync.dma_start(
                    out_d[b, cot * 128 : (cot + 1) * 128, h0 : h0 + rows, :],
                    ot.rearrange("p (h w) -> p h w", h=rows),
                )

            def conv_pgroup(b, cot, p):
                pc = psump.tile([128, PIX_ROWS * W], F32, tag="ps", name="ps")
                first = True
                for c in range(NCH):
                    for term in range(3):
                        for kk in range(CHK * c, CHK * (c + 1)):
                            last = c == NCH - 1 and term == 2 and kk == KK - 1
                            mm(pc, b, cot, p, kk, term, start=first, stop=last)
                            first = False
                evict(pc, b, cot, p)

            # ---- warmup: 6 garbage matmuls ramp the PE p-state ----
            for i in range(6):
                wps = cpsump.tile([128, CCH], F32, tag="cps", name="cps")
                nc.tensor.matmul(
                    wps[:],
                    wtile[:, 0:128],
                    wtile[:, 0:CCH],
                    start=True,
                    stop=True,
                )

            # ---- (b0, cot0): chunk-passes over pixel groups 0-3 ----
            # PE combines chunks c0 (both ct), c1/ct1, c2 (both ct); the DVE
            # takes c1/ct0 via an FMA chain plus b1's products as banks land.
            def cpass(pcs, c, p_range):
                for p in p_range:
                    for term in range(3):
                        for kk in range(CHK * c, CHK * (c + 1)):
                            mm(
                                pcs[p], 0, 0, p, kk, term,
                                start=(c == 0 and term == 0 and kk == 0),
                                stop=(c == NCH - 1 and term == 2 and kk == KK - 1),
                            )

            dve_muls(1, 0, 0, 0, 1)
            pcs = {
                p: psump.tile([128, PIX_ROWS * W], F32, tag="ps", name="ps")
                for p in range(NP_PASS)
            }
            pe_combine(0, 0)
            pe_combine(1, 0)
            dve_chunk_stt_chain(0, 0, 0, 1)  # c1/ct0 on the DVE
            cpass(pcs, 0, range(0, 2))
            pe_combine(1, 1)  # c1/ct1
            dve_chunk_stt_splits(0, 0, 0, 1)
            cpass(pcs, 0, range(2, NP_PASS))
            dve_muls(1, 0, 0, 1, 2)
            cpass(pcs, 1, range(0, NP_PASS))
            dve_muls(1, 0, 0, 2, 3)
            pe_combine(0, 2)
            pe_combine(1, 2)
            cpass(pcs, 2, range(0, NP_PASS))
            for p in range(NP_PASS):
                evict(pcs[p], 0, 0, p)

            # b1/cot0 combine finish, interleaved into pixel groups 4-7
            # so both w8(b1) splits land before the (b1, cot0) group starts
            dve_adds(0)
            dve_muls(1, 1, 0, 0, NCH)
            act_w8_split(1, 0, 0)
            dve_w8r_split(1, 0, 0)
            conv_pgroup(0, 0, 4)
            conv_pgroup(0, 0, 5)
            dve_adds(1)
            act_w8_split(1, 1, 0)
            dve_w8r_split(1, 1, 0)
            conv_pgroup(0, 0, 6)
            conv_pgroup(0, 0, 7)

            for p in range(NP_PASS):
                conv_pgroup(1, 0, p)
            dve_unit(0, 0, 1)
            conv_pgroup(1, 0, 4)
            conv_pgroup(1, 0, 5)
            dve_unit(0, 1, 1)
            conv_pgroup(1, 0, 6)
            conv_pgroup(1, 0, 7)

            for p in range(NP_PASS):
                conv_pgroup(0, 1, p)
            dve_unit(1, 0, 1)
            conv_pgroup(0, 1, 4)
            conv_pgroup(0, 1, 5)
            dve_unit(1, 1, 1)
            conv_pgroup(0, 1, 6)
            conv_pgroup(0, 1, 7)

            # last group: final pixel group split in two 4-row halves to
            # shorten the eviction/DMA tail after the last matmul
            for p in range(NPG - 1):
                conv_pgroup(1, 1, p)
            for half in range(2):
                h0 = (NPG - 1) * PIX_ROWS + 4 * half
                pc = psump.tile([128, 4 * W], F32, tag="ps", name="ps")
                first = True
                for c in range(NCH):
                    for term in range(3):
                        for kk in range(CHK * c, CHK * (c + 1)):
                            last = c == NCH - 1 and term == 2 and kk == KK - 1
                            mm(pc, 1, 1, NPG - 1, kk, term, start=first, stop=last,
                               rows=4, h0=h0)
                            first = False
                evict(pc, 1, 1, NPG - 1, rows=4, h0=h0)
    nc.compile()
    return nc


def kernel(x, routing_weights, expert_weight):
    global LAST_RESULTS
    x = np.asarray(x, dtype=np.float32)
    r = np.asarray(routing_weights, dtype=np.float32)
    bank = np.asarray(expert_weight, dtype=np.float32)
    fp8 = ml_dtypes.float8_e4m3

    # x: [B, 256, H, W] -> padded per-ct layout [B, 128, ct, HP, WP],
    # split into fp8 value + fp8 residual (host-side quantization).
    xp = np.zeros((B, 128, CT, HP, WP), dtype=np.float32)
    xp[:, :, :, 1 : H + 1, 1 : W + 1] = x.reshape(B, CT, 128, H, W).transpose(0, 2, 1, 3, 4)
    x8 = xp.astype(fp8)
    x8r = (xp - x8.astype(np.float32)).astype(fp8)
    x8 = x8.reshape(B, 128, CT * HP * WP)
    x8r = x8r.reshape(B, 128, CT * HP * WP)

    # bank: [E, co*ci*kh*kw] -> [cot, ct, chunk, e, ci, (3 taps, co)] bf16
    bk = bank.reshape(E, CO_T, 128, CT, 128, NCH, CHK) * W_SCALE
    bank_t = np.ascontiguousarray(
        bk.transpose(1, 3, 5, 0, 4, 6, 2).reshape(CO_T, CT, NCH, E, 128, CCH)
    ).astype(ml_dtypes.bfloat16)

    if not _NC_CACHE:
        _NC_CACHE.append(_build())
    nc = _NC_CACHE[0]

    in_maps = []
    for c in range(N_CORES):
        rows = r[c * BPC : (c + 1) * BPC].reshape(BPC * E)
        rbc = np.ascontiguousarray(np.broadcast_to(rows[None, :], (128, BPC * E)))
        in_maps.append(
            {
                "x8": np.ascontiguousarray(x8[c * BPC : (c + 1) * BPC]),
                "x8r": np.ascontiguousarray(x8r[c * BPC : (c + 1) * BPC]),
                "bank": bank_t,
                "routb": rbc.astype(ml_dtypes.bfloat16),
            }
        )

    trace = bool(os.environ.get("KERNEL_TRACE"))
    try:
        res = run_bass_kernel_spmd(
            nc, in_maps, core_ids=list(range(N_CORES)), trace=trace
        )
    except ModuleNotFoundError:
        if not trace:
            raise
        # Tracing unavailable in this environment (no axon NTFF hook).
        res = run_bass_kernel_spmd(
            nc, in_maps, core_ids=list(range(N_CORES)), trace=False
        )
    LAST_RESULTS = res
    return np.concatenate([rr["out"] for rr in res.results], axis=0)
